# revision 18
# baseline (speedup 1.0000x reference)
"""Trainium2 Bass kernel for Custom_RoPE (rotate-half RoPE + per-(head,token)
min/max observer).

Reference computation (float branch):
    out = x * cos + rotate_half(x) * sin        # (H, T, D)
    obs_max = max(out, axis=-1)                 # (H, T)
    obs_min = min(out, axis=-1)

Sharding: tokens (T) are split across the 8 NeuronCores (1024 tokens each).
All math is independent per (head, token), so no communication is needed, and
T-sharding also shards the cos/sin reads (vs. duplicating them 8x under
head-sharding).

Per-core dataflow.  SBUF tiles hold KT=4 consecutive tokens per partition (so
HBM DMA descriptors are KT*D*4 = 2 KiB, not 512 B) and (head, token, d) on
the free axis.  The rotate-half sign flip is pre-baked into an "sm" tile
(-sin for d<64, +sin for d>=64) built once on ScalarE, so

    out = x*cos + swap(x)*sm .

Engine assignment (hardware-measured constraints: a DVE op with BOTH sources
in SBUF serializes against GpSimd on the shared SBUF port, but a DVE op whose
second source is in PSUM uses the separate PSUM read port and runs fully
concurrent with GpSimd; tensor_reduce is 1-src and also concurrent):

  - cos/sm for the current block are copied into PSUM by ScalarE.
  - The 32 heads of each token block are split: GP_HEADS on GpSimd
    (mult/mult-halves/add, all SBUF operands), the rest on VectorE with every
    2-src op reading its second operand from PSUM -> zero contention.
  - Both observer reductions (1-src) always run on VectorE.
  - Loads go on the Sync HWDGE queue, stores on the ScalarE HWDGE queue, so
    stores never block loads; the Q7 cores never generate DMA descriptors.
  - Emission order per engine: VectorE does its own waves' elementwise work
    first and the GpSimd waves' reductions last, so it never head-of-line
    stalls waiting for GpSimd.
"""

import numpy as np

import concourse.bacc as bacc
import concourse.mybir as mybir
from concourse import bass_utils
from concourse.tile import TileContext

H, T, D = 32, 8192, 128
NCORES = 8
TL = T // NCORES  # tokens per core
P = 128  # SBUF partitions
KT = 4  # consecutive tokens per partition
TB = P * KT  # tokens per block (512)
NBLK = TL // TB  # blocks per core (2)
HALF = D // 2
F32 = mybir.dt.float32

GP_HEADS = 16  # heads per block on GpSimd (multiple of WH_G)
WH_G = 8  # heads per GpSimd wave (bigger waves amortize Q7 sem overhead)
WH_V = 4  # heads per VectorE wave

_CACHE = {}


def _build(gp_heads=GP_HEADS):
    nc = bacc.Bacc("TRN2", target_bir_lowering=False, debug=False, num_devices=NCORES)
    x = nc.dram_tensor("x", (H, TL, D), F32, kind="ExternalInput")
    cos = nc.dram_tensor("cos", (TL, D), F32, kind="ExternalInput")
    sin = nc.dram_tensor("sin", (TL, D), F32, kind="ExternalInput")
    out = nc.dram_tensor("out", (H, TL, D), F32, kind="ExternalOutput")
    # Token-major observer outputs; host transposes to (H, TL).
    omax = nc.dram_tensor("omax", (TL, H), F32, kind="ExternalOutput")
    omin = nc.dram_tensor("omin", (TL, H), F32, kind="ExternalOutput")

    mult = mybir.AluOpType.mult
    add = mybir.AluOpType.add

    def split_waves(lo, hi, wh):
        waves = []
        h0 = lo
        while h0 < hi:
            waves.append((h0, min(h0 + wh, hi)))
            h0 += wh
        return waves

    gp_waves = split_waves(0, gp_heads, WH_G)
    dve_waves = split_waves(gp_heads, H, WH_V)

    with TileContext(nc) as tc:
        with (
            tc.tile_pool(name="const", bufs=1) as constp,
            tc.tile_pool(name="io", bufs=3) as io,
            tc.tile_pool(name="tmp", bufs=2) as tmp,
            tc.tile_pool(name="psum", bufs=1, space="PSUM") as psp,
        ):
            # cos / sin / sm for the whole core: (P, NBLK, KT, D)
            ct_all = constp.tile([P, NBLK, KT, D], F32)
            st_all = constp.tile([P, NBLK, KT, D], F32)
            sm_all = constp.tile([P, NBLK, KT, D], F32)
            for b in range(NBLK):
                bs = slice(b * TB, (b + 1) * TB)
                nc.sync.dma_start(
                    ct_all[:, b].rearrange("p k d -> p (k d)"),
                    cos.ap()[bs, :].rearrange("(p k) d -> p (k d)", p=P),
                )
                nc.sync.dma_start(
                    st_all[:, b].rearrange("p k d -> p (k d)"),
                    sin.ap()[bs, :].rearrange("(p k) d -> p (k d)", p=P),
                )
            nc.scalar.mul(sm_all[:, :, :, 0:HALF], st_all[:, :, :, 0:HALF], -1.0)
            nc.scalar.copy(sm_all[:, :, :, HALF:D], st_all[:, :, :, HALF:D])

            # PSUM copies of the constants for both blocks (ScalarE), so the
            # block-1 copies never queue behind block-0 stores.
            c_ps, sm_ps = {}, {}
            for b in range(NBLK):
                c_ps[b] = psp.tile(
                    [P, KT * D], F32, tag="c", bufs=NBLK, name=f"cps_{b}"
                )
                sm_ps[b] = psp.tile(
                    [P, KT, D], F32, tag="s", bufs=NBLK, name=f"smps_{b}"
                )
                nc.scalar.copy(c_ps[b][:, :], ct_all[:, b].rearrange("p k d -> p (k d)"))
                nc.scalar.copy(sm_ps[b][:, :, :], sm_all[:, b])

            # persistent observer accumulators: (P, NBLK, KT, H), h contiguous
            omax_t = constp.tile([P, NBLK, KT, H], F32)
            omin_t = constp.tile([P, NBLK, KT, H], F32)

            xws, ots = {}, {}
            NV_TOT = NBLK * len(dve_waves)
            NG_TOT = NBLK * len(gp_waves)
            # GP-wave reductions are woven into the VectorE stream at the
            # V-wave indices where the GpSimd adds should have completed.
            red_after = {}
            for j in range(NG_TOT):
                pos = min((j + 1) * NV_TOT // max(NG_TOT, 1) + 1, NV_TOT)
                red_after.setdefault(pos, []).append(j)
            gp_red_args = []
            emitted_reds = set()
            v_emitted = 0

            def reds_and_store(b, h0, h1):
                bs_ = slice(b * TB, (b + 1) * TB)
                ot = ots[(b, h0)]
                otr = ot.rearrange("p h (k d) -> p (h k) d", d=D)
                mx = omax_t[:, b, :, h0:h1].rearrange("p k h -> p h k")
                mn = omin_t[:, b, :, h0:h1].rearrange("p k h -> p h k")
                nc.vector.tensor_reduce(
                    mx, otr, axis=mybir.AxisListType.X, op=mybir.AluOpType.max
                )
                nc.vector.tensor_reduce(
                    mn, otr, axis=mybir.AxisListType.X, op=mybir.AluOpType.min
                )
                nc.scalar.dma_start(
                    out.ap()[h0:h1, bs_, :].rearrange(
                        "h (p k) d -> p h (k d)", p=P
                    ),
                    ot[:, :, :],
                )

            for b in range(NBLK):
                bs = slice(b * TB, (b + 1) * TB)
                cb = ct_all[:, b].rearrange("p k d -> p (k d)")  # (P, KT*D) SBUF
                smb = sm_all[:, b]  # (P, KT, D) SBUF

                # interleave loads, GpSimd's first wave first (it is the
                # longer-running engine and has no other work to hide behind)
                load_order = []
                for i in range(max(len(dve_waves), len(gp_waves))):
                    if i < len(gp_waves):
                        load_order.append(("g", gp_waves[i]))
                    if i < len(dve_waves):
                        load_order.append(("v", dve_waves[i]))

                for kind, (h0, h1) in load_order:
                    wh = h1 - h0
                    xw = io.tile(
                        [P, wh, KT * D], F32, tag=f"x{kind}", bufs=(2 if kind == "g" else 3), name=f"xw_{b}_{h0}"
                    )
                    nc.sync.dma_start(
                        xw[:, :, :],
                        x.ap()[h0:h1, bs, :].rearrange("h (p k) d -> p h (k d)", p=P),
                    )
                    xws[(b, h0)] = xw

                for h0, h1 in gp_waves:
                    wh = h1 - h0
                    xw = xws[(b, h0)]
                    xw4 = xw.rearrange("p h (k d) -> p h k d", d=D)
                    ot = io.tile([P, wh, KT * D], F32, tag="og", bufs=2, name=f"ot_{b}_{h0}")
                    ots[(b, h0)] = ot
                    gp_red_args.append((b, h0, h1))
                    t1 = tmp.tile([P, wh, KT * D], F32, tag="t1g", name=f"t1_{b}_{h0}")
                    t2 = tmp.tile([P, wh, KT, D], F32, tag="t2g", name=f"t2_{b}_{h0}")
                    nc.gpsimd.tensor_tensor(
                        t1[:, :, :],
                        xw[:, :, :],
                        cb[:, None, :].broadcast_to((P, wh, KT * D)),
                        mult,
                    )
                    nc.gpsimd.tensor_tensor(
                        t2[:, :, :, 0:HALF],
                        xw4[:, :, :, HALF:D],
                        smb[:, None, :, 0:HALF].broadcast_to((P, wh, KT, HALF)),
                        mult,
                    )
                    nc.gpsimd.tensor_tensor(
                        t2[:, :, :, HALF:D],
                        xw4[:, :, :, 0:HALF],
                        smb[:, None, :, HALF:D].broadcast_to((P, wh, KT, HALF)),
                        mult,
                    )
                    nc.gpsimd.tensor_tensor(
                        ot[:, :, :],
                        t1[:, :, :],
                        t2.rearrange("p h k d -> p h (k d)"),
                        add,
                    )

                for h0, h1 in dve_waves:
                    wh = h1 - h0
                    xw = xws[(b, h0)]
                    xw4 = xw.rearrange("p h (k d) -> p h k d", d=D)
                    ot = io.tile([P, wh, KT * D], F32, tag="ov", name=f"ot_{b}_{h0}")
                    ots[(b, h0)] = ot
                    # VectorE wave: 2nd src always PSUM -> no contention.
                    t1p = psp.tile(
                        [P, wh, KT * D], F32, tag="t1p", name=f"t1p_{b}_{h0}"
                    )
                    t2 = tmp.tile([P, wh, KT, D], F32, tag="t2v", name=f"t2_{b}_{h0}")
                    nc.vector.tensor_tensor(
                        t1p[:, :, :],
                        xw[:, :, :],
                        c_ps[b][:, None, :].broadcast_to((P, wh, KT * D)),
                        mult,
                    )
                    nc.vector.tensor_tensor(
                        t2[:, :, :, 0:HALF],
                        xw4[:, :, :, HALF:D],
                        sm_ps[b][:, None, :, 0:HALF].broadcast_to((P, wh, KT, HALF)),
                        mult,
                    )
                    nc.vector.tensor_tensor(
                        t2[:, :, :, HALF:D],
                        xw4[:, :, :, 0:HALF],
                        sm_ps[b][:, None, :, HALF:D].broadcast_to((P, wh, KT, HALF)),
                        mult,
                    )
                    nc.vector.tensor_tensor(
                        ot[:, :, :],
                        t2.rearrange("p h k d -> p h (k d)"),
                        t1p[:, :, :],
                        add,
                    )
                    reds_and_store(b, h0, h1)
                    v_emitted += 1
                    for j in red_after.get(v_emitted, []):
                        if j < len(gp_red_args):
                            emitted_reds.add(j)
                            reds_and_store(*gp_red_args[j])

            for j in range(len(gp_red_args)):
                if j not in emitted_reds:
                    reds_and_store(*gp_red_args[j])

            for b in range(NBLK):
                bs = slice(b * TB, (b + 1) * TB)
                nc.sync.dma_start(
                    omax.ap()[bs, :].rearrange("(p k) h -> p k h", p=P),
                    omax_t[:, b, :, :],
                )
                nc.sync.dma_start(
                    omin.ap()[bs, :].rearrange("(p k) h -> p k h", p=P),
                    omin_t[:, b, :, :],
                )

    nc.compile()
    return nc


def get_nc(gp_heads=GP_HEADS):
    if gp_heads not in _CACHE:
        _CACHE[gp_heads] = _build(gp_heads)
    return _CACHE[gp_heads]


def kernel(x, scale_x, cos, scale_cos, sin, scale_sin, **run_kwargs):
    x = np.asarray(x, dtype=np.float32)
    cos = np.asarray(cos, dtype=np.float32)
    sin = np.asarray(sin, dtype=np.float32)
    assert x.shape == (H, T, D), x.shape

    nc = get_nc()
    in_maps = []
    for c in range(NCORES):
        ts = slice(c * TL, (c + 1) * TL)
        in_maps.append(
            {
                "x": np.ascontiguousarray(x[:, ts, :]),
                "cos": np.ascontiguousarray(cos[ts, :]),
                "sin": np.ascontiguousarray(sin[ts, :]),
            }
        )
    res = bass_utils.run_bass_kernel_spmd(
        nc, in_maps, core_ids=list(range(NCORES)), **run_kwargs
    )
    out = np.concatenate([res.results[c]["out"] for c in range(NCORES)], axis=1)
    obs_max = np.concatenate(
        [res.results[c]["omax"].T for c in range(NCORES)], axis=1
    )
    obs_min = np.concatenate(
        [res.results[c]["omin"].T for c in range(NCORES)], axis=1
    )
    kernel.last_results = res
    return out, obs_max, obs_min


# revision 20
# speedup vs baseline: 1.1670x; 1.1670x over previous
"""Trainium2 Bass kernel for Custom_RoPE (rotate-half RoPE + per-(head,token)
min/max observer).

Reference computation (float branch):
    out = x * cos + rotate_half(x) * sin        # (H, T, D)
    obs_max = max(out, axis=-1)                 # (H, T)
    obs_min = min(out, axis=-1)

Sharding: tokens (T) are split across the 8 NeuronCores (1024 tokens each).
All math is independent per (head, token), so no communication is needed, and
T-sharding also shards the cos/sin reads (vs. duplicating them 8x under
head-sharding).

Per-core dataflow.  SBUF tiles hold KT=4 consecutive tokens per partition (so
HBM DMA descriptors are KT*D*4 = 2 KiB, not 512 B) and (head, token, d) on
the free axis.  The rotate-half sign flip is pre-baked into an "sm" tile
(-sin for d<64, +sin for d>=64) built once on ScalarE, so

    out = x*cos + swap(x)*sm .

Engine assignment (hardware-measured constraints: a DVE op with BOTH sources
in SBUF serializes against GpSimd on the shared SBUF port, but a DVE op whose
second source is in PSUM uses the separate PSUM read port and runs fully
concurrent with GpSimd; tensor_reduce is 1-src and also concurrent):

  - cos/sm for the current block are copied into PSUM by ScalarE.
  - The 32 heads of each token block are split: GP_HEADS on GpSimd
    (mult/mult-halves/add, all SBUF operands), the rest on VectorE with every
    2-src op reading its second operand from PSUM -> zero contention.
  - Both observer reductions (1-src) always run on VectorE.
  - Loads go on the Sync HWDGE queue, stores on the ScalarE HWDGE queue, so
    stores never block loads; the Q7 cores never generate DMA descriptors.
  - Emission order per engine: VectorE does its own waves' elementwise work
    first and the GpSimd waves' reductions last, so it never head-of-line
    stalls waiting for GpSimd.
"""

import numpy as np

import concourse.bacc as bacc
import concourse.mybir as mybir
from concourse import bass_utils
from concourse.tile import TileContext

H, T, D = 32, 8192, 128
NCORES = 8
TL = T // NCORES  # tokens per core
P = 128  # SBUF partitions
KT = 4  # consecutive tokens per partition
TB = P * KT  # tokens per block (512)
NBLK = TL // TB  # blocks per core (2)
HALF = D // 2
F32 = mybir.dt.float32

GP_HEADS = 16  # heads per block on GpSimd (multiple of WH_G)
WH_G = 8  # heads per GpSimd wave (bigger waves amortize Q7 sem overhead)
WH_V = 4  # heads per VectorE wave

_CACHE = {}


def _build(gp_heads=GP_HEADS):
    nc = bacc.Bacc("TRN2", target_bir_lowering=False, debug=False, num_devices=NCORES)
    x = nc.dram_tensor("x", (H, TL, D), F32, kind="ExternalInput")
    cos = nc.dram_tensor("cos", (TL, D), F32, kind="ExternalInput")
    sin = nc.dram_tensor("sin", (TL, D), F32, kind="ExternalInput")
    out = nc.dram_tensor("out", (H, TL, D), F32, kind="ExternalOutput")
    # Token-major observer outputs; host transposes to (H, TL).
    omax = nc.dram_tensor("omax", (TL, H), F32, kind="ExternalOutput")
    omin = nc.dram_tensor("omin", (TL, H), F32, kind="ExternalOutput")

    mult = mybir.AluOpType.mult
    add = mybir.AluOpType.add

    def split_waves(lo, hi, wh):
        waves = []
        h0 = lo
        while h0 < hi:
            waves.append((h0, min(h0 + wh, hi)))
            h0 += wh
        return waves

    gp_waves = split_waves(0, gp_heads, WH_G)
    dve_waves = split_waves(gp_heads, H, WH_V)

    with TileContext(nc) as tc:
        with (
            tc.tile_pool(name="const", bufs=1) as constp,
            tc.tile_pool(name="io", bufs=3) as io,
            tc.tile_pool(name="tmp", bufs=2) as tmp,
            tc.tile_pool(name="psum", bufs=1, space="PSUM") as psp,
        ):
            # cos / sin / sm for the whole core: (P, NBLK, KT, D)
            ct_all = constp.tile([P, NBLK, KT, D], F32)
            st_all = constp.tile([P, NBLK, KT, D], F32)
            sm_all = constp.tile([P, NBLK, KT, D], F32)
            for b in range(NBLK):
                bs = slice(b * TB, (b + 1) * TB)
                nc.sync.dma_start(
                    ct_all[:, b].rearrange("p k d -> p (k d)"),
                    cos.ap()[bs, :].rearrange("(p k) d -> p (k d)", p=P),
                )
                nc.sync.dma_start(
                    st_all[:, b].rearrange("p k d -> p (k d)"),
                    sin.ap()[bs, :].rearrange("(p k) d -> p (k d)", p=P),
                )
            nc.scalar.mul(sm_all[:, :, :, 0:HALF], st_all[:, :, :, 0:HALF], -1.0)
            nc.scalar.copy(sm_all[:, :, :, HALF:D], st_all[:, :, :, HALF:D])

            # PSUM copies of the constants for both blocks (ScalarE), so the
            # block-1 copies never queue behind block-0 stores.
            c_ps, sm_ps = {}, {}
            for b in range(NBLK):
                c_ps[b] = psp.tile(
                    [P, KT * D], F32, tag="c", bufs=NBLK, name=f"cps_{b}"
                )
                sm_ps[b] = psp.tile(
                    [P, KT, D], F32, tag="s", bufs=NBLK, name=f"smps_{b}"
                )
                nc.scalar.copy(c_ps[b][:, :], ct_all[:, b].rearrange("p k d -> p (k d)"))
                nc.scalar.copy(sm_ps[b][:, :, :], sm_all[:, b])

            # persistent observer accumulators: (P, NBLK, KT, H), h contiguous
            omax_t = constp.tile([P, NBLK, KT, H], F32)
            omin_t = constp.tile([P, NBLK, KT, H], F32)

            xws, ots = {}, {}
            NV_TOT = NBLK * len(dve_waves)
            NG_TOT = NBLK * len(gp_waves)
            # GP-wave reductions are woven into the VectorE stream at the
            # V-wave indices where the GpSimd adds should have completed.
            red_after = {}
            for j in range(NG_TOT):
                pos = min((j + 1) * NV_TOT // max(NG_TOT, 1) + 1, NV_TOT)
                red_after.setdefault(pos, []).append(j)
            gp_red_args = []
            emitted_reds = set()
            v_emitted = 0

            def reds_and_store(b, h0, h1):
                bs_ = slice(b * TB, (b + 1) * TB)
                ot = ots[(b, h0)]
                otr = ot.rearrange("p h (k d) -> p (h k) d", d=D)
                mx = omax_t[:, b, :, h0:h1].rearrange("p k h -> p h k")
                mn = omin_t[:, b, :, h0:h1].rearrange("p k h -> p h k")
                nc.vector.tensor_reduce(
                    mx, otr, axis=mybir.AxisListType.X, op=mybir.AluOpType.max
                )
                nc.vector.tensor_reduce(
                    mn, otr, axis=mybir.AxisListType.X, op=mybir.AluOpType.min
                )
                nc.scalar.dma_start(
                    out.ap()[h0:h1, bs_, :].rearrange(
                        "h (p k) d -> p h (k d)", p=P
                    ),
                    ot[:, :, :],
                )

            for b in range(NBLK):
                bs = slice(b * TB, (b + 1) * TB)
                cb = ct_all[:, b].rearrange("p k d -> p (k d)")  # (P, KT*D) SBUF
                smb = sm_all[:, b]  # (P, KT, D) SBUF

                # interleave loads, GpSimd's first wave first (it is the
                # longer-running engine and has no other work to hide behind)
                load_order = []
                for i in range(max(len(dve_waves), len(gp_waves))):
                    if i < len(gp_waves):
                        load_order.append(("g", gp_waves[i]))
                    if i < len(dve_waves):
                        load_order.append(("v", dve_waves[i]))

                for kind, (h0, h1) in load_order:
                    wh = h1 - h0
                    xw = io.tile(
                        [P, wh, KT * D], F32, tag=f"x{kind}", bufs=(2 if kind == "g" else 3), name=f"xw_{b}_{h0}"
                    )
                    nc.sync.dma_start(
                        xw[:, :, :],
                        x.ap()[h0:h1, bs, :].rearrange("h (p k) d -> p h (k d)", p=P),
                    )
                    xws[(b, h0)] = xw

                for h0, h1 in gp_waves:
                    wh = h1 - h0
                    xw = xws[(b, h0)]
                    xw4 = xw.rearrange("p h (k d) -> p h k d", d=D)
                    ot = io.tile([P, wh, KT * D], F32, tag="og", bufs=2, name=f"ot_{b}_{h0}")
                    ots[(b, h0)] = ot
                    gp_red_args.append((b, h0, h1))
                    t1 = tmp.tile([P, wh, KT * D], F32, tag="t1g", name=f"t1_{b}_{h0}")
                    t2 = tmp.tile([P, wh, KT, D], F32, tag="t2g", name=f"t2_{b}_{h0}")
                    nc.gpsimd.tensor_tensor(
                        t1[:, :, :],
                        xw[:, :, :],
                        cb[:, None, :].broadcast_to((P, wh, KT * D)),
                        mult,
                    )
                    nc.gpsimd.tensor_tensor(
                        t2[:, :, :, 0:HALF],
                        xw4[:, :, :, HALF:D],
                        smb[:, None, :, 0:HALF].broadcast_to((P, wh, KT, HALF)),
                        mult,
                    )
                    nc.gpsimd.tensor_tensor(
                        t2[:, :, :, HALF:D],
                        xw4[:, :, :, 0:HALF],
                        smb[:, None, :, HALF:D].broadcast_to((P, wh, KT, HALF)),
                        mult,
                    )
                    nc.gpsimd.tensor_tensor(
                        ot[:, :, :],
                        t1[:, :, :],
                        t2.rearrange("p h k d -> p h (k d)"),
                        add,
                    )

                for h0, h1 in dve_waves:
                    wh = h1 - h0
                    xw = xws[(b, h0)]
                    xw4 = xw.rearrange("p h (k d) -> p h k d", d=D)
                    ot = io.tile([P, wh, KT * D], F32, tag="ov", name=f"ot_{b}_{h0}")
                    ots[(b, h0)] = ot
                    # VectorE wave: 2nd src always PSUM -> no contention.
                    t1p = psp.tile(
                        [P, wh, KT * D], F32, tag="t1p", name=f"t1p_{b}_{h0}"
                    )
                    t2 = tmp.tile([P, wh, KT, D], F32, tag="t2v", name=f"t2_{b}_{h0}")
                    nc.vector.tensor_tensor(
                        t1p[:, :, :],
                        xw[:, :, :],
                        c_ps[b][:, None, :].broadcast_to((P, wh, KT * D)),
                        mult,
                    )
                    nc.vector.tensor_tensor(
                        t2[:, :, :, 0:HALF],
                        xw4[:, :, :, HALF:D],
                        sm_ps[b][:, None, :, 0:HALF].broadcast_to((P, wh, KT, HALF)),
                        mult,
                    )
                    nc.vector.tensor_tensor(
                        t2[:, :, :, HALF:D],
                        xw4[:, :, :, 0:HALF],
                        sm_ps[b][:, None, :, HALF:D].broadcast_to((P, wh, KT, HALF)),
                        mult,
                    )
                    nc.vector.tensor_tensor(
                        ot[:, :, :],
                        t2.rearrange("p h k d -> p h (k d)"),
                        t1p[:, :, :],
                        add,
                    )
                    reds_and_store(b, h0, h1)
                    v_emitted += 1
                    for j in red_after.get(v_emitted, []):
                        if j < len(gp_red_args):
                            emitted_reds.add(j)
                            reds_and_store(*gp_red_args[j])

            for j in range(len(gp_red_args)):
                if j not in emitted_reds:
                    reds_and_store(*gp_red_args[j])

            for b in range(NBLK):
                bs = slice(b * TB, (b + 1) * TB)
                nc.sync.dma_start(
                    omax.ap()[bs, :].rearrange("(p k) h -> p k h", p=P),
                    omax_t[:, b, :, :],
                )
                nc.sync.dma_start(
                    omin.ap()[bs, :].rearrange("(p k) h -> p k h", p=P),
                    omin_t[:, b, :, :],
                )

    nc.compile()
    return nc




def _build_pe3():
    """All-wave-uniform variant: DVE = TT1 (PSUM-src cos) + reductions,
    GpSimd = swap-mults only, TensorE = add (identity matmuls, fp32-exact),
    ScalarE = PSUM->SBUF copies + stores.  DVE stream software-pipelined one
    wave (reds emitted one wave late) to avoid head-of-line stalls."""
    from concourse.masks import make_identity

    nc = bacc.Bacc("TRN2", target_bir_lowering=False, debug=False, num_devices=NCORES)
    x = nc.dram_tensor("x", (H, TL, D), F32, kind="ExternalInput")
    cos = nc.dram_tensor("cos", (TL, D), F32, kind="ExternalInput")
    sin = nc.dram_tensor("sin", (TL, D), F32, kind="ExternalInput")
    out = nc.dram_tensor("out", (H, TL, D), F32, kind="ExternalOutput")
    omax = nc.dram_tensor("omax", (TL, H), F32, kind="ExternalOutput")
    omin = nc.dram_tensor("omin", (TL, H), F32, kind="ExternalOutput")
    mult = mybir.AluOpType.mult
    WH = 4
    waves = [(h0, h0 + WH) for h0 in range(0, H, WH)]

    with TileContext(nc) as tc:
        with (
            tc.tile_pool(name="const", bufs=1) as constp,
            tc.tile_pool(name="io", bufs=4) as io,
            tc.tile_pool(name="tmp", bufs=3) as tmp,
            tc.tile_pool(name="psum", bufs=1, space="PSUM") as psp,
        ):
            ident = constp.tile([P, P], F32)
            make_identity(nc, ident)
            ct_all = constp.tile([P, NBLK, KT, D], F32)
            st_all = constp.tile([P, NBLK, KT, D], F32)
            sm_all = constp.tile([P, NBLK, KT, D], F32)
            for b in range(NBLK):
                bs = slice(b * TB, (b + 1) * TB)
                nc.sync.dma_start(
                    ct_all[:, b].rearrange("p k d -> p (k d)"),
                    cos.ap()[bs, :].rearrange("(p k) d -> p (k d)", p=P),
                )
                nc.sync.dma_start(
                    st_all[:, b].rearrange("p k d -> p (k d)"),
                    sin.ap()[bs, :].rearrange("(p k) d -> p (k d)", p=P),
                )
            nc.scalar.mul(sm_all[:, :, :, 0:HALF], st_all[:, :, :, 0:HALF], -1.0)
            nc.scalar.copy(sm_all[:, :, :, HALF:D], st_all[:, :, :, HALF:D])

            c_ps = {}
            for b in range(NBLK):
                c_ps[b] = psp.tile([P, KT * D], F32, tag="c", bufs=NBLK, name=f"cps_{b}")
                nc.scalar.copy(c_ps[b][:, :], ct_all[:, b].rearrange("p k d -> p (k d)"))

            omax_t = constp.tile([P, NBLK, KT, H], F32)
            omin_t = constp.tile([P, NBLK, KT, H], F32)

            pending = []  # (b, h0, h1, ot) reds one wave late

            def emit_reds(b, h0, h1, ot):
                otr = ot.rearrange("p h (k d) -> p (h k) d", d=D)
                mx = omax_t[:, b, :, h0:h1].rearrange("p k h -> p h k")
                mn = omin_t[:, b, :, h0:h1].rearrange("p k h -> p h k")
                nc.vector.tensor_reduce(
                    mx, otr, axis=mybir.AxisListType.X, op=mybir.AluOpType.max
                )
                nc.vector.tensor_reduce(
                    mn, otr, axis=mybir.AxisListType.X, op=mybir.AluOpType.min
                )

            for b in range(NBLK):
                bs = slice(b * TB, (b + 1) * TB)
                smb = sm_all[:, b]
                for h0, h1 in waves:
                    wh = h1 - h0
                    xw = io.tile([P, wh, KT * D], F32, tag="x", name=f"xw_{b}_{h0}")
                    nc.sync.dma_start(
                        xw[:, :, :],
                        x.ap()[h0:h1, bs, :].rearrange("h (p k) d -> p h (k d)", p=P),
                    )
                    xw4 = xw.rearrange("p h (k d) -> p h k d", d=D)
                    t1 = tmp.tile([P, wh, KT * D], F32, tag="t1", name=f"t1_{b}_{h0}")
                    t2 = tmp.tile([P, wh, KT, D], F32, tag="t2", name=f"t2_{b}_{h0}")
                    nc.vector.tensor_tensor(
                        t1[:, :, :],
                        xw[:, :, :],
                        c_ps[b][:, None, :].broadcast_to((P, wh, KT * D)),
                        mult,
                    )
                    nc.gpsimd.tensor_tensor(
                        t2[:, :, :, 0:HALF],
                        xw4[:, :, :, HALF:D],
                        smb[:, None, :, 0:HALF].broadcast_to((P, wh, KT, HALF)),
                        mult,
                    )
                    nc.gpsimd.tensor_tensor(
                        t2[:, :, :, HALF:D],
                        xw4[:, :, :, 0:HALF],
                        smb[:, None, :, HALF:D].broadcast_to((P, wh, KT, HALF)),
                        mult,
                    )
                    t2f = t2.rearrange("p h k d -> p h (k d)")
                    ot = io.tile([P, wh, KT * D], F32, tag="o", name=f"ot_{b}_{h0}")
                    for c0 in range(0, wh, 2):
                        ps = psp.tile(
                            [P, 2, KT * D], F32, tag="pso", bufs=3,
                            name=f"ps_{b}_{h0}_{c0}",
                        )
                        for i in range(2):
                            nc.tensor.matmul(
                                ps[:, i, :], ident[:, :], t1[:, c0 + i, :],
                                start=True, stop=False,
                            )
                            nc.tensor.matmul(
                                ps[:, i, :], ident[:, :], t2f[:, c0 + i, :],
                                start=False, stop=True,
                            )
                        nc.scalar.copy(ot[:, c0 : c0 + 2, :], ps[:, :, :])
                    nc.scalar.dma_start(
                        out.ap()[h0:h1, bs, :].rearrange("h (p k) d -> p h (k d)", p=P),
                        ot[:, :, :],
                    )
                    pending.append((b, h0, h1, ot))
                    if len(pending) > 1:
                        emit_reds(*pending.pop(0))
            while pending:
                emit_reds(*pending.pop(0))

            for b in range(NBLK):
                bs = slice(b * TB, (b + 1) * TB)
                nc.sync.dma_start(
                    omax.ap()[bs, :].rearrange("(p k) h -> p k h", p=P),
                    omax_t[:, b, :, :],
                )
                nc.sync.dma_start(
                    omin.ap()[bs, :].rearrange("(p k) h -> p k h", p=P),
                    omin_t[:, b, :, :],
                )
    nc.compile()
    return nc


def get_nc_pe3():
    if "pe3" not in _CACHE:
        _CACHE["pe3"] = _build_pe3()
    return _CACHE["pe3"]



def get_nc(gp_heads=GP_HEADS):
    if gp_heads not in _CACHE:
        _CACHE[gp_heads] = _build(gp_heads)
    return _CACHE[gp_heads]


def kernel(x, scale_x, cos, scale_cos, sin, scale_sin, **run_kwargs):
    x = np.asarray(x, dtype=np.float32)
    cos = np.asarray(cos, dtype=np.float32)
    sin = np.asarray(sin, dtype=np.float32)
    assert x.shape == (H, T, D), x.shape

    nc = get_nc_pe3()
    in_maps = []
    for c in range(NCORES):
        ts = slice(c * TL, (c + 1) * TL)
        in_maps.append(
            {
                "x": np.ascontiguousarray(x[:, ts, :]),
                "cos": np.ascontiguousarray(cos[ts, :]),
                "sin": np.ascontiguousarray(sin[ts, :]),
            }
        )
    res = bass_utils.run_bass_kernel_spmd(
        nc, in_maps, core_ids=list(range(NCORES)), **run_kwargs
    )
    out = np.concatenate([res.results[c]["out"] for c in range(NCORES)], axis=1)
    obs_max = np.concatenate(
        [res.results[c]["omax"].T for c in range(NCORES)], axis=1
    )
    obs_min = np.concatenate(
        [res.results[c]["omin"].T for c in range(NCORES)], axis=1
    )
    kernel.last_results = res
    return out, obs_max, obs_min
